# revision 1
# baseline (speedup 1.0000x reference)
"""Fused multi-head bilinear attention (softmax over query axis m) on 8 trn2 cores.

Reference computation (b=2, m=n=2048, e=128, k=8, d=16):
    r   = einsum('bmp,kpd->bmkd', x, lambda1) + bias_lambda
    A   = einsum('bmkd,kqd,bnq->kbmn', r, lambda2, y) * d**-0.5
    att = softmax(A, axis=m)
    r2  = einsum('kbmn,bmp,kpd->bnkd', att, x, theta1) + bias_theta
    out = einsum('bnkd,kqd->bnq', r2, theta2)

Sharding: 8 cores = 2 batches x 4 n-quarters (512 wide). Each core computes all 8
heads for its output slice out[b, nq*512:(nq+1)*512, :]; unshard is pure concat.

Per-core pipeline (all heads):
  X^T, Y^T arrive host-pre-transposed; R^T = (X@L1)^T and S^T = (Y@L2)^T with
  heads packed into 32-partition strips (16 used + 16 zero) so K=16 matmuls are
  32-aligned (f32r via rounded copies of X^T/Y^T and the lambdas);
  T = X@theta1 stored [m, (head, 33)]: 16 data cols, 16 zeros, and a ones col at
  32 per head.  Per head: A tiles [m128, 512] = R^T.T @ S^T (f32r), exp on
  ScalarE straight out of PSUM in 2048/1024-wide calls, then U[33, n] +=
  TAUG_k^T @ expA accumulated over m -- row 32 is the softmax denominator for
  free (the ones column).  U rows 0:16 are normalized in place by a reciprocal
  row broadcast across partitions (GpSimd partition_broadcast), giving
  r2^T[(k d), n] strip-packed directly as the final lhsT; out = r2^T.T @
  theta2^T contracts all 128 (k,d) rows at full PE width (zero half-strips on
  both sides keep the sum exact).  U-matmuls are emitted with a lag behind
  A/exp so the PE never head-of-line blocks on the exp of its own group.
"""

import sys

from contextlib import ExitStack

import numpy as np

try:
    import concourse.bass as bass
except ImportError:
    sys.path.append("/opt/trn_rl_repo")
    import concourse.bass as bass
import concourse.tile as tile
from concourse import bacc, mybir
from concourse.bass import ds, ts
from concourse.masks import make_identity

F32 = mybir.dt.float32
F32R = mybir.dt.float32r
EXP = mybir.ActivationFunctionType.Exp

B, M, N, E, K, D = 2, 2048, 2048, 128, 8, 16
NCORES = 8
NSLICE = N // 4          # n columns per core (one batch, quarter of n)
MT = M // 128            # 16 m-tiles
SCALE = float(D) ** -0.5
# m-tile groups for A/exp staging: (start, len) in units of 512-wide m-tiles.
# Groups alternate between two 3-bank PSUM pools; 6 groups per head keeps the
# alternation seamless across head boundaries (no same-pool adjacency).
GROUPS = [(0, 3), (3, 3), (6, 3), (9, 3), (12, 2), (14, 2)]


def _emit(tc: tile.TileContext, ctx: ExitStack, io: dict):
    nc = tc.nc
    xb, ybs, l1g, l2g, t1a, t2t, blg, btb, outb = (
        io["xb"], io["ybs"], io["l1g"], io["l2g"], io["t1a"], io["t2t"],
        io["blg"], io["btb"], io["outb"],
    )

    const = ctx.enter_context(tc.tile_pool(name="const", bufs=1))
    persist = ctx.enter_context(tc.tile_pool(name="persist", bufs=1))
    expa3_pool = ctx.enter_context(tc.tile_pool(name="expa3", bufs=3))
    expa2_pool = ctx.enter_context(tc.tile_pool(name="expa2", bufs=3))
    den_pool = ctx.enter_context(tc.tile_pool(name="den", bufs=4))
    out_pool = ctx.enter_context(tc.tile_pool(name="outp", bufs=2))
    ps_pa = ctx.enter_context(tc.tile_pool(name="ps_pa", bufs=1, space="PSUM"))
    ps_pb = ctx.enter_context(tc.tile_pool(name="ps_pb", bufs=1, space="PSUM"))
    ps_u = ctx.enter_context(tc.tile_pool(name="ps_u", bufs=2, space="PSUM"))

    pp = [0]

    def ping(shape):
        # strict global alternation between the two 3-bank PSUM staging pools
        pp[0] ^= 1
        pool, tag = (ps_pa, "pa") if pp[0] else (ps_pb, "pb")
        return pool.tile(shape, F32, tag=tag, name="pro%s" % tag)

    # ---- parameter loads -------------------------------------------------
    L1 = const.tile([128, 2, 128], F32)   # strip-packed lambda1 per head-group
    L2 = const.tile([128, 2, 128], F32)
    T1A = const.tile([128, 128], F32)     # theta1 packed (k d)
    T2T = const.tile([128, 2, 128], F32)  # strip-packed theta2^T per group
    BLG = const.tile([128, 2], F32)       # strip-packed bias_lambda
    BTC = const.tile([128, 2], F32)       # strip-packed bias_theta
    XT = persist.tile([128, M], F32, name="XT")       # [e, m]
    YT = persist.tile([128, NSLICE], F32, name="YT")  # [e, n]
    nc.sync.dma_start(YT[:], ybs)
    nc.sync.dma_start(XT[:, 0:512], xb[:, 0:512])
    for g in range(2):
        nc.sync.dma_start(L1[:, g, :], l1g[g])
        nc.sync.dma_start(L2[:, g, :], l2g[g])
    nc.sync.dma_start(T1A[:], t1a)
    for g in range(2):
        nc.sync.dma_start(T2T[:, g, :], t2t[g])
    nc.sync.dma_start(BLG[:], blg)
    nc.sync.dma_start(BTC[:], btb)

    # ---- persistent intermediates ---------------------------------------
    # X^T / Y^T arrive pre-transposed from the host (layout prep), plain f32;
    # the projection matmuls consuming them run fp32 and their PSUM
    # evacuations round into f32r tiles.
    ident = const.tile([128, 128], F32)
    make_identity(nc, ident[:])
    # dummy transposes keep the PE busy (and ramp its p-state) while the
    # first input DMAs are still in flight
    for _w in range(28):
        wp = ping([128, 128])
        nc.tensor.transpose(wp[:], ident[:], ident[:])
    XTR = persist.tile([128, M], F32R)     # f32r copies feed the projections
    YTR = persist.tile([128, NSLICE], F32R)
    RT = persist.tile([128, 2, M], F32R)       # R^T strips [32h+j, g, m]
    ST = persist.tile([128, 2, NSLICE], F32R)  # S^T strips
    # per head 33 lhsT columns: 16 of X@theta1, 16 zeros, ones at 32 so the
    # denominator lands on a 32-aligned U row
    TAUG = persist.tile([128, MT, K * 33], F32R)
    R2TG = persist.tile([128, 2, NSLICE], F32)  # strip-packed [(h d) g n]
    ONES = const.tile([128, MT * K], F32)
    nc.gpsimd.memset(ONES[:], 1.0)
    ZEROS = const.tile([128, MT * K * 16], F32)
    nc.gpsimd.memset(ZEROS[:], 0.0)
    nc.gpsimd.memset(R2TG[:], 0.0)
    nc.vector.tensor_copy(
        TAUG[:].rearrange("p mt (k s) -> p mt k s", k=K)[:, :, :, 32:33],
        ONES[:].rearrange("p (mt k) -> p mt k", k=K)[:, :, :, None])
    nc.vector.tensor_copy(
        TAUG[:].rearrange("p mt (k s) -> p mt k s", k=K)[:, :, :, 16:32],
        ZEROS[:].rearrange("p (mt k d) -> p mt k d", k=K, d=16))
    L1R = const.tile([128, 2, 128], F32R)
    L2R = const.tile([128, 2, 128], F32R)
    nc.vector.tensor_copy(L1R[:], L1[:])
    nc.vector.tensor_copy(L2R[:], L2[:])

    def y_block():
        ps = ping([128, NSLICE])
        nc.tensor.matmul(ps[:], lhsT=L2[:, 0, :], rhs=YT[:], start=True, stop=True)
        nc.vector.tensor_copy(ST[:, 0, :], ps[:])
        nc.vector.tensor_copy(YTR[:], YT[:])

    def q4_block(q4):
        if q4 > 0:
            nc.sync.dma_start(XT[:, ts(q4, 512)], xb[:, ts(q4, 512)])
        ps = ping([128, 512])
        nc.tensor.matmul(ps[:], lhsT=L1[:, 0, :], rhs=XT[:, ts(q4, 512)],
                         start=True, stop=True)
        nc.vector.tensor_scalar_add(RT[:, 0, ts(q4, 512)], ps[:], BLG[:, 0:1])
        nc.vector.tensor_copy(XTR[:, ts(q4, 512)], XT[:, ts(q4, 512)])
        for j in range(4):
            mt = q4 * 4 + j
            ps = ping([128, 128])
            nc.tensor.matmul(ps[:], lhsT=XT[:, ts(mt, 128)], rhs=T1A[:],
                             start=True, stop=True)
            nc.vector.tensor_copy(
                TAUG[:, mt, :].rearrange("p (k s) -> p k s", k=K)[:, :, 0:16],
                ps[:].rearrange("p (k d) -> p k d", k=K))

    # U accumulators are [33, n]: rows 0-15 numerator, row 32 denominator

    def rs_g1_block():
        # group-1 projections in two wide tiles: back-to-back matmuls with a
        # single evacuation each, so the pipeline is not head-of-line blocked
        # by a slot->evac->slot chain when this pops mid-stream
        ps = ping([128, 1536])
        for c in range(3):
            nc.tensor.matmul(ps[:, ts(c, 512)], lhsT=L1R[:, 1, :],
                             rhs=XTR[:, ts(c, 512)], start=True, stop=True)
        nc.vector.tensor_scalar_add(RT[:, 1, 0:1536], ps[:], BLG[:, 1:2])
        ps = ping([128, 1024])
        nc.tensor.matmul(ps[:, 0:512], lhsT=L1R[:, 1, :], rhs=XTR[:, ts(3, 512)],
                         start=True, stop=True)
        nc.tensor.matmul(ps[:, 512:1024], lhsT=L2R[:, 1, :], rhs=YTR[:],
                         start=True, stop=True)
        nc.vector.tensor_scalar_add(RT[:, 1, ts(3, 512)], ps[:, 0:512],
                                    BLG[:, 1:2])
        nc.vector.tensor_copy(ST[:, 1, :], ps[:, 512:1024])

    # ---- head pipeline: U-matmuls emitted with a lag ---------------------
    LAG = 3
    pending = []

    def flush(limit):
        while len(pending) > limit:
            pending.pop(0)()

    def mk_ubatch(U, k, mst, glen, expa):
        def emit():
            for j in range(glen):
                mt = mst + j
                nc.tensor.matmul(
                    U[:], lhsT=TAUG[:, mt, ds(33 * k, 33)],
                    rhs=expa[:, ts(j, 512)],
                    start=(mt == 0), stop=(mt == MT - 1))
        return emit

    def mk_finalize(U, k, split=False):
        g, h = divmod(k, 4)
        strip = 32 * h

        def emit():
            den = den_pool.tile([1, NSLICE], F32, tag="den", name="den")
            nc.vector.reciprocal(den[:], U[32:33, :])
            rb = den_pool.tile([16, NSLICE], F32, tag="rb", name="rb")
            nc.gpsimd.partition_broadcast(rb[:], den[:])
            # for the last head, normalize chunk-by-chunk so the output
            # matmuls can start on chunk 0 before the whole row is done
            chunks = [ts(c, 128) for c in range(NSLICE // 128)] if split \
                else [slice(0, NSLICE)]
            for sl in chunks:
                nc.vector.tensor_mul(
                    R2TG[strip:strip + 16, g, sl], U[0:16, sl], rb[:, sl])
                nc.vector.tensor_scalar_add(
                    R2TG[strip:strip + 16, g, sl],
                    R2TG[strip:strip + 16, g, sl],
                    BTC[strip:strip + 16, g:g + 1])
        return emit

    heads_state = {}

    def head_group(k, gi):
        g, h = divmod(k, 4)
        strip = 32 * h
        if gi == 0:
            heads_state[k] = ps_u.tile([33, NSLICE], F32, tag="u", name="U")
        U = heads_state[k]
        mst, glen = GROUPS[gi]
        aps = ping([128, 512 * glen])
        for j in range(glen):
            mt = mst + j
            nc.tensor.matmul(
                aps[:, ts(j, 512)],
                lhsT=RT[strip:strip + 16, g, ds(mt * 128, 128)],
                rhs=ST[strip:strip + 16, g, :],
                start=True, stop=True, tile_position=(strip, 0))
        epool = expa3_pool if glen == 3 else expa2_pool
        expa = epool.tile([128, 512 * glen], F32R, tag="e%d" % glen, name="expa")
        nc.scalar.activation(expa[:], aps[:], EXP, scale=SCALE)
        pending.append(mk_ubatch(U, k, mst, glen, expa))
        flush(LAG)
        if gi == len(GROUPS) - 1:
            pending.append(mk_finalize(U, k, split=(k == K - 1)))

    # prologue interleaved with heads 0-1 (group gi needs RT chunks <= its mts)
    y_block()
    q4_block(0)
    head_group(0, 0)
    head_group(1, 0)
    q4_block(1)
    head_group(0, 1)
    head_group(1, 1)
    q4_block(2)
    head_group(0, 2)
    head_group(1, 2)
    head_group(0, 3)
    head_group(1, 3)
    q4_block(3)
    head_group(0, 4)
    head_group(1, 4)
    head_group(0, 5)
    head_group(1, 5)
    pending.insert(0, rs_g1_block)
    for k in range(2, K):
        for gi in range(len(GROUPS)):
            head_group(k, gi)
    flush(0)

    # ---- output: out[n, q] = (r2 + bias_theta) @ theta2^T ---------------
    # r2 and theta2^T are strip-packed with zeros in the unused half-strips,
    # so accumulating both groups' full-K matmuls gives the exact sum over kd.
    OB = out_pool.tile([128, NSLICE // 128, 128], F32, tag="ob")
    for ch in range(NSLICE // 128):
        op = ping([128, 128])
        for g in range(2):
            nc.tensor.matmul(op[:], lhsT=R2TG[:, g, ts(ch, 128)], rhs=T2T[:, g, :],
                             start=(g == 0), stop=(g == 1))
        nc.vector.tensor_copy(OB[:, ch, :], op[:])
        if ch == 1:
            nc.sync.dma_start(
                outb[0:256, :].rearrange("(c p) q -> p c q", p=128), OB[:, 0:2, :])
    nc.sync.dma_start(
        outb[256:512, :].rearrange("(c p) q -> p c q", p=128), OB[:, 2:4, :])


_CACHE = {}


def build():
    if "nc" in _CACHE:
        return _CACHE["nc"]
    nc = bacc.Bacc("TRN2", target_bir_lowering=False, debug=False,
                   num_devices=NCORES)
    io = {
        "xb": nc.dram_tensor("xb", [E, M], F32, kind="ExternalInput").ap(),
        "ybs": nc.dram_tensor("ybs", [E, NSLICE], F32, kind="ExternalInput").ap(),
        "l1g": nc.dram_tensor("l1g", [2, E, 128], F32, kind="ExternalInput").ap(),
        "l2g": nc.dram_tensor("l2g", [2, E, 128], F32, kind="ExternalInput").ap(),
        "t1a": nc.dram_tensor("t1a", [E, 128], F32, kind="ExternalInput").ap(),
        "t2t": nc.dram_tensor("t2t", [2, 128, E], F32, kind="ExternalInput").ap(),
        "blg": nc.dram_tensor("blg", [128, 2], F32, kind="ExternalInput").ap(),
        "btb": nc.dram_tensor("btb", [128, 2], F32, kind="ExternalInput").ap(),
        "outb": nc.dram_tensor("outb", [NSLICE, E], F32, kind="ExternalOutput").ap(),
    }
    with tile.TileContext(nc) as tc:
        with ExitStack() as ctx:
            _emit(tc, ctx, io)
    nc.compile()
    _CACHE["nc"] = nc
    return nc


def make_in_maps(x, y, lambda1, lambda2, theta1, theta2, bias_lambda, bias_theta):
    f = np.float32
    l1g = np.zeros((2, E, 128), f)
    l2g = np.zeros((2, E, 128), f)
    t2t = np.zeros((2, 128, E), f)
    blg = np.zeros((128, 2), f)
    btb = np.zeros((128, 2), f)
    for g in range(2):
        for h in range(4):
            l1g[g, :, 32 * h:32 * h + 16] = lambda1[4 * g + h]
            l2g[g, :, 32 * h:32 * h + 16] = lambda2[4 * g + h]
            t2t[g, 32 * h:32 * h + 16, :] = theta2[4 * g + h].T
            blg[32 * h:32 * h + 16, g] = bias_lambda[4 * g + h]
            btb[32 * h:32 * h + 16, g] = bias_theta[4 * g + h]
    t1a = np.ascontiguousarray(theta1.transpose(1, 0, 2).reshape(E, K * D))
    xts = [np.ascontiguousarray(np.asarray(x[b], dtype=f).T) for b in range(B)]
    maps = []
    for c in range(NCORES):
        b, q = divmod(c, 4)
        maps.append({
            "xb": xts[b],
            "ybs": np.ascontiguousarray(
                np.asarray(y[b, q * NSLICE:(q + 1) * NSLICE], dtype=f).T),
            "l1g": l1g, "l2g": l2g, "t1a": t1a, "t2t": t2t,
            "blg": blg, "btb": btb,
        })
    return maps


def kernel(x, y, lambda1, lambda2, theta1, theta2, bias_lambda, bias_theta):
    from concourse.bass_utils import run_bass_kernel_spmd
    nc = build()
    maps = make_in_maps(x, y, lambda1, lambda2, theta1, theta2,
                        bias_lambda, bias_theta)
    res = run_bass_kernel_spmd(nc, maps, list(range(NCORES)))
    out = np.empty((B, N, E), np.float32)
    for c in range(NCORES):
        b, q = divmod(c, 4)
        out[b, q * NSLICE:(q + 1) * NSLICE] = res.results[c]["outb"]
    return out



# revision 66
# speedup vs baseline: 1.5271x; 1.5271x over previous
"""Fused multi-head bilinear attention (softmax over query axis m) on 8 trn2 cores.

Reference computation (b=2, m=n=2048, e=128, k=8, d=16):
    r   = einsum('bmp,kpd->bmkd', x, lambda1) + bias_lambda
    A   = einsum('bmkd,kqd,bnq->kbmn', r, lambda2, y) * d**-0.5
    att = softmax(A, axis=m)
    r2  = einsum('kbmn,bmp,kpd->bnkd', att, x, theta1) + bias_theta
    out = einsum('bnkd,kqd->bnq', r2, theta2)

Sharding: 8 cores = 2 batches x 4 n-quarters (512 wide).  Each core computes all
8 heads for its output slice out[b, nq*512:(nq+1)*512, :]; unshard is pure
concat.

Per-core pipeline (all heads):
  X^T (f32r + bf16 copies), Y^T arrive host-pre-transposed.  R^T/S^T
  projections run in f32r (strip-packed per head group); T = X@theta1 is
  bf16, stored [m, (k, 17)] with a ones column at 16 per head so the softmax
  denominator falls out of the U accumulation for free.  Per head: 8 m-groups
  of A tiles [m128x2, 512] = R^T.T @ S^T (f32r, 1 cyc/col) rotate across
  THREE 2-bank PSUM pools so the A(g) -> exp(g) -> A(g+3) staging chain never
  blocks the PE.  exp is split across TWO engines: ScalarE runs the true exp
  (activation, bf16 out) for half the groups, and the DVE computes a
  Schraudolph-style approximation for the other half -- a single
  tensor_scalar (A*c1 + c2) written as int16 whose bit pattern IS the bf16
  exp(A) (relative ripple ~2%, which largely cancels in the softmax ratio).
  U accumulation is TRANSPOSED vs the naive form: UT[n128, 17] += expa.T @
  TAUG[mt,k] so each matmul streams only 17 columns (bf16, 1 cyc/col) instead
  of 512 -- 3.9x less PE time for the whole attention*V product.  The four
  n-tile accumulators live in ONE psum bank per head parity, accumulated
  manually (start=False + a 1-contract PE matmul against a zeros row to
  clear them) so no psum zero-region groups are ever opened.  Finalize per
  head: a single broadcast-divide by the ones column (DVE) into NORMALL;
  tail: PE transpose back to [kd, n], bias_theta added on evac, and one
  full-width (kd=128) bf16 matmul against theta2^T per output chunk, with
  the four output DMAs spread across SP/Pool/Act queues.
"""

import sys

from contextlib import ExitStack

import numpy as np

try:
    import concourse.bass as bass
except ImportError:
    sys.path.append("/opt/trn_rl_repo")
    import concourse.bass as bass
import concourse.tile as tile
from concourse import bacc, mybir
from concourse.bass import ds, ts
from concourse.masks import make_identity

F32 = mybir.dt.float32
F32R = mybir.dt.float32r
BF16 = mybir.dt.bfloat16
I16 = mybir.dt.int16
EXP = mybir.ActivationFunctionType.Exp
MULT = mybir.AluOpType.mult
ADD = mybir.AluOpType.add

B, M, N, E, K, D = 2, 2048, 2048, 128, 8, 16
NCORES = 8
NSLICE = N // 4          # n columns per core (one batch, quarter of n)
MT = M // 128            # 16 m-tiles
NT = NSLICE // 128       # 4 n-tiles
SCALE = float(D) ** -0.5
LN2 = float(np.log(2.0))
SCH_C = 6.0              # Schraudolph offset, tuned end-to-end
SCH_C1 = SCALE * 128.0 / LN2
SCH_C2 = 127.0 * 128.0 - SCH_C

# m-tile groups for A/exp staging: (start, len) in units of 128-row m-tiles.
# Groups rotate across THREE 2-bank PSUM pools: the A(g) -> exp(g) ->
# A(g+depth) chain is 3 deep, so the PE never waits on the exp of the
# group it just produced.  U accumulation is manual (no start/stop groups),
# so all four n-tile accumulators fit in ONE psum bank.
GROUPS = [(2 * i, 2) for i in range(8)]
# exp engine per (head, group): 's' = ScalarE true exp, 'v' = DVE Schraudolph.
import os

EXP_PAT = os.environ.get("KEXP", "svsvsvsv")


def exp_eng(k, gi):
    pats = EXP_PAT.split(",")
    return pats[k % len(pats)][gi]


WARMUP = int(os.environ.get("KWARMUP", "18"))
LAG = int(os.environ.get("KLAG", "6"))
UZERO = os.environ.get("KUZERO", "p")
IDENT = mybir.ActivationFunctionType.Identity
COPY = mybir.ActivationFunctionType.Copy


def _emit(tc: tile.TileContext, ctx: ExitStack, io: dict):
    nc = tc.nc
    xtf, xtb, ytf, pfr, pfb, pbh, outb = (
        io["xtf"], io["xtb"], io["ytf"], io["pfr"], io["pfb"], io["pbh"],
        io["outb"])

    const = ctx.enter_context(tc.tile_pool(name="const", bufs=1))
    persist = ctx.enter_context(tc.tile_pool(name="persist", bufs=1))
    expa_pool = ctx.enter_context(
        tc.tile_pool(name="expa", bufs=int(os.environ.get("KEBUF", "12"))))
    fin_pool = ctx.enter_context(tc.tile_pool(name="fin", bufs=2))
    out_pool = ctx.enter_context(tc.tile_pool(name="outp", bufs=1))
    ps_pools = [
        ctx.enter_context(tc.tile_pool(name="ps_p%d" % i, bufs=1,
                                       space="PSUM"))
        for i in range(3)
    ]
    ps_u = ctx.enter_context(tc.tile_pool(name="ps_u", bufs=1, space="PSUM"))

    pp = [0]

    def ping(shape, dtype=F32):
        # rotate across the three 2-bank staging pools
        pp[0] = (pp[0] + 1) % 3
        return ps_pools[pp[0]].tile(shape, dtype, tag="p%d" % pp[0],
                                    name="prop%d" % pp[0])

    # ---- parameter + data loads -----------------------------------------
    # f32r matmul inputs must come from f32r-typed buffers end-to-end (the
    # BIR verifier requires producers to round to f32r), so the x/y/lambda
    # tensors are declared float32r all the way from DRAM.
    PFR = const.tile([128, 512], F32R)  # l1g | l2g
    PFB = const.tile([128, 3], F32)     # blg | btkd
    PB = const.tile([128, 256], BF16)   # t1ab | t2tb
    XTF = persist.tile([128, M], F32R, name="XTF")
    XTB = persist.tile([128, M], BF16, name="XTB")
    YTF = persist.tile([128, NSLICE], F32R, name="YTF")
    nc.scalar.dma_start(PFR[:], pfr)
    nc.scalar.dma_start(PFB[:], pfb)
    nc.sync.dma_start(XTF[:, 0:512], xtf[:, 0:512])
    nc.sync.dma_start(YTF[:], ytf)
    nc.sync.dma_start(XTB[:, 0:512], xtb[:, 0:512])
    nc.sync.dma_start(PB[:], pbh)
    nc.sync.dma_start(XTF[:, 512:1024], xtf[:, 512:1024])
    nc.sync.dma_start(XTB[:, 512:1024], xtb[:, 512:1024])
    nc.sync.dma_start(XTF[:, 1024:1536], xtf[:, 1024:1536])
    nc.sync.dma_start(XTB[:, 1024:1536], xtb[:, 1024:1536])
    nc.sync.dma_start(XTF[:, 1536:2048], xtf[:, 1536:2048])
    nc.sync.dma_start(XTB[:, 1536:2048], xtb[:, 1536:2048])

    def L1G(g):
        return PFR[:, ts(g, 128)]

    def L2G(g):
        return PFR[:, ds(256 + g * 128, 128)]

    BLG = PFB[:, 0:2]             # strip-packed bias_lambda (f32)
    BTKD = PFB[:, 2:3]            # bias_theta, kd-major (f32)
    T1AB = PB[:, 0:128]           # theta1 packed (k d), bf16
    T2TB = PB[:, 128:256]         # theta2^T packed [kd, q], bf16

    # ---- persistent intermediates ---------------------------------------
    RT = persist.tile([128, 2, M], F32R)       # R^T strips [32h+j, g, m]
    ST = persist.tile([128, 2, NSLICE], F32R)  # S^T strips
    # per head 17 rhs columns: 16 of X@theta1 and ones at 16 (denominator)
    TAUG = persist.tile([128, MT, K, 17], BF16)
    R2T = persist.tile([128, NSLICE], BF16)    # normalized r2^T [(k d), n]
    nc.gpsimd.memset(TAUG[:, :, :, 16:17], 1.0)

    ZROW = const.tile([1, 128], BF16)
    nc.vector.memset(ZROW[:], 0.0)
    ident = const.tile([128, 128], F32)
    make_identity(nc, ident[:])
    identr = ident[:]
    # dummy transposes keep the PE busy (and ramp its p-state) while the
    # first input DMAs are in flight
    for _w in range(WARMUP):
        wp = ping([128, 128])
        nc.tensor.matmul(wp[:], lhsT=identr, rhs=identr, is_transpose=True,
                         start=True, stop=True)

    # U accumulators: all four n-tiles in ONE psum bank, accumulated
    # manually (start=False always + DVE pre-zeroing) so no psum
    # zero-region groups are ever opened in these banks.  Double-buffered
    # across head parity so head k+1 accumulates while head k finalizes.
    U4 = [ps_u.tile([128, 512], F32, tag="u4%d" % i, name="U4")
          for i in range(2)]
    UVS = [u[:, 0:NT * 17].rearrange("p (t s) -> p t s", s=17) for u in U4]
    NORMALL = persist.tile([128, NT, 128], F32, name="NORMALL")
    nc.vector.memset(UVS[0], 0.0)
    nc.vector.memset(UVS[1], 0.0)

    def y_block():
        ps = ping([128, NSLICE])
        nc.tensor.matmul(ps[:], lhsT=L2G(0), rhs=YTF[:],
                         start=True, stop=True)
        nc.vector.tensor_copy(ST[:, 0, :], ps[:])

    def q4_block(q4):
        ps = ping([128, 512])
        nc.tensor.matmul(ps[:], lhsT=L1G(0), rhs=XTF[:, ts(q4, 512)],
                         start=True, stop=True)
        # RT evac + bias on the Scalar engine (Identity keeps DVE free)
        nc.scalar.activation(RT[:, 0, ts(q4, 512)], ps[:], IDENT,
                             bias=BLG[:, 0:1])
        for j in range(2):
            ps = ping([128, 256])
            for i in range(2):
                mt = q4 * 4 + 2 * j + i
                nc.tensor.matmul(ps[:, ts(i, 128)],
                                 lhsT=XTB[:, ts(mt, 128)], rhs=T1AB,
                                 start=True, stop=True)
            nc.scalar.activation(
                TAUG[:, ds(q4 * 4 + 2 * j, 2), :, 0:16],
                ps[:].rearrange("p (m k d) -> p m k d", m=2, k=K), COPY)

    def rs_g1_block():
        # group-1 projections: two 1024-wide tiles + one 512-wide
        for c2 in range(2):
            ps = ping([128, 1024])
            for c in range(2):
                nc.tensor.matmul(ps[:, ts(c, 512)], lhsT=L1G(1),
                                 rhs=XTF[:, ts(2 * c2 + c, 512)],
                                 start=True, stop=True)
            nc.scalar.activation(RT[:, 1, ts(c2, 1024)], ps[:], IDENT,
                                 bias=BLG[:, 1:2])
        ps = ping([128, 512])
        nc.tensor.matmul(ps[:], lhsT=L2G(1), rhs=YTF[:],
                         start=True, stop=True)
        nc.vector.tensor_copy(ST[:, 1, :], ps[:])

    # ---- head pipeline: U-matmuls emitted with a lag ---------------------
    pending = []

    def flush(limit):
        while len(pending) > limit:
            pending.pop(0)()

    def mk_ubatch(k, mst, glen, expa):
        uv = UVS[k % 2]

        def emit():
            for j in range(glen):
                mt = mst + j
                for t in range(NT):
                    nc.tensor.matmul(
                        uv[:, t, :],
                        lhsT=expa[:, ds(j * 512 + t * 128, 128)],
                        rhs=TAUG[:, mt, k, :],
                        start=False, stop=False, skip_group_check=True)
        return emit

    def mk_fin_head(k):
        uv = UVS[k % 2]

        def emit():
            # (a DVE op may read only ONE psum input, so the reciprocal goes
            # through SBUF first)
            rden = fin_pool.tile([128, NT], F32, tag="rden", name="rden")
            nc.vector.reciprocal(rden[:], uv[:, :, 16])
            nc.vector.tensor_tensor(
                NORMALL[:, :, ds(k * 16, 16)],
                uv[:, :, 0:16],
                rden[:, :, None].broadcast_to([128, NT, 16]),
                MULT)
            if UZERO == 's':
                nc.scalar.memzero(uv)
            elif UZERO == 'p':
                # zero the U region with a 1-contract matmul against a zeros
                # row: 68 cycles on the PE, which has plenty of slack
                nc.tensor.matmul(uv.rearrange("p t s -> p (t s)"),
                                 lhsT=ZROW[:], rhs=ZROW[0:1, 0:NT * 17],
                                 start=True, stop=True)
            else:
                nc.vector.memset(uv, 0.0)
        return emit

    def head_group(k, gi):
        if k == K - 1 and gi == 1:
            # pull fin(K-2) ahead of head K-1's exps in the DVE queue so it
            # is not left for the tail
            flush(1)
        g, h = divmod(k, 4)
        strip = 32 * h
        mst, glen = GROUPS[gi]
        aps = ping([128, 512 * glen])
        for j in range(glen):
            mt = mst + j
            nc.tensor.matmul(
                aps[:, ts(j, 512)],
                lhsT=RT[strip:strip + 16, g, ds(mt * 128, 128)],
                rhs=ST[strip:strip + 16, g, :],
                start=True, stop=True, tile_position=(strip, 0))
        expa = expa_pool.tile([128, 512 * glen], BF16, tag="ex", name="expa")
        if exp_eng(k, gi) == 's':
            nc.scalar.activation(expa[:], aps[:], EXP, scale=SCALE)
        else:
            nc.vector.tensor_scalar(expa[:].bitcast(I16), aps[:],
                                    SCH_C1, SCH_C2, MULT, ADD)
        pending.append(mk_ubatch(k, mst, glen, expa))
        flush(LAG)
        if gi == len(GROUPS) - 1:
            pending.append(mk_fin_head(k))

    def tail_out():
        # out-matmuls go to the (now free) U banks so they never contend
        # with the transposes' staging pools; output DMAs spread across the
        # three DMA-capable engines
        dma_eng = [nc.sync, nc.gpsimd, nc.sync, nc.scalar]
        for t in range(NT):
            pst = ping([128, 128])
            nc.tensor.matmul(pst[:], lhsT=NORMALL[:, t, :], rhs=identr,
                             is_transpose=True, start=True, stop=True)
            if t % 2 == 0:
                nc.scalar.activation(R2T[:, ts(t, 128)], pst[:], IDENT,
                                     bias=BTKD)
            else:
                nc.vector.tensor_scalar_add(R2T[:, ts(t, 128)], pst[:], BTKD)
            ops = U4[t % 2][:, 0:128]
            nc.tensor.matmul(ops, lhsT=R2T[:, ts(t, 128)], rhs=T2TB,
                             start=True, stop=True)
            ob = out_pool.tile([128, 128], F32, tag="ob%d" % t, name="ob")
            if t % 2 == 0:
                nc.vector.tensor_copy(ob[:], ops)
            else:
                nc.scalar.activation(ob[:], ops, COPY)
            dma_eng[t].dma_start(outb[ts(t, 128), :], ob[:])

    # heads run in even/odd PAIRS (each parity has its own U bank), giving
    # two independent A/exp streams to hide DMA waits and pool-chain latency
    y_block()
    q4_block(0)
    head_group(0, 0)
    head_group(1, 0)
    head_group(0, 1)
    head_group(1, 1)
    q4_block(1)
    head_group(0, 2)
    head_group(1, 2)
    head_group(0, 3)
    head_group(1, 3)
    q4_block(2)
    head_group(0, 4)
    head_group(1, 4)
    head_group(0, 5)
    head_group(1, 5)
    q4_block(3)
    head_group(0, 6)
    head_group(1, 6)
    head_group(0, 7)
    head_group(1, 7)
    pending.insert(0, rs_g1_block)
    for kp in range(2, K, 2):
        for gi in range(len(GROUPS)):
            head_group(kp, gi)
            head_group(kp + 1, gi)
    flush(0)
    tail_out()


_CACHE = {}


def build():
    if "nc" in _CACHE:
        return _CACHE["nc"]
    nc = bacc.Bacc("TRN2", target_bir_lowering=False, debug=False,
                   num_devices=NCORES)
    io = {
        "xtf": nc.dram_tensor("xtf", [E, M], F32R, kind="ExternalInput").ap(),
        "xtb": nc.dram_tensor("xtb", [E, M], BF16, kind="ExternalInput").ap(),
        "ytf": nc.dram_tensor("ytf", [E, NSLICE], F32R,
                              kind="ExternalInput").ap(),
        "pfr": nc.dram_tensor("pfr", [128, 512], F32R,
                              kind="ExternalInput").ap(),
        "pfb": nc.dram_tensor("pfb", [128, 3], F32,
                              kind="ExternalInput").ap(),
        "pbh": nc.dram_tensor("pbh", [128, 256], BF16,
                              kind="ExternalInput").ap(),
        "outb": nc.dram_tensor("outb", [NSLICE, E], F32,
                               kind="ExternalOutput").ap(),
    }
    with tile.TileContext(nc) as tc:
        with ExitStack() as ctx:
            _emit(tc, ctx, io)
    nc.compile()
    _CACHE["nc"] = nc
    return nc


def make_in_maps(x, y, lambda1, lambda2, theta1, theta2, bias_lambda,
                 bias_theta):
    import ml_dtypes
    f = np.float32
    bf = ml_dtypes.bfloat16
    pfr = np.zeros((128, 512), f)
    pfb = np.zeros((128, 3), f)
    for g in range(2):
        for h in range(4):
            k = 4 * g + h
            pfr[:, g * 128 + 32 * h: g * 128 + 32 * h + 16] = lambda1[k]
            pfr[:, 256 + g * 128 + 32 * h: 256 + g * 128 + 32 * h + 16] = \
                lambda2[k]
            pfb[32 * h:32 * h + 16, g] = bias_lambda[k]
    pfb[:, 2] = np.asarray(bias_theta, f).reshape(128)
    pbh = np.zeros((128, 256), bf)
    pbh[:, 0:128] = np.ascontiguousarray(
        theta1.transpose(1, 0, 2).reshape(E, K * D)).astype(bf)
    pbh[:, 128:256] = np.ascontiguousarray(
        theta2.transpose(0, 2, 1).reshape(K * D, E)).astype(bf)
    maps = []
    for c in range(NCORES):
        b, q = divmod(c, 4)
        xt = np.ascontiguousarray(np.asarray(x[b], dtype=f).T)
        maps.append({
            "xtf": xt,
            "xtb": xt.astype(bf),
            "ytf": np.ascontiguousarray(
                np.asarray(y[b, q * NSLICE:(q + 1) * NSLICE], dtype=f).T),
            "pfr": pfr, "pfb": pfb, "pbh": pbh,
        })
    return maps


def kernel(x, y, lambda1, lambda2, theta1, theta2, bias_lambda, bias_theta):
    from concourse.bass_utils import run_bass_kernel_spmd
    nc = build()
    maps = make_in_maps(x, y, lambda1, lambda2, theta1, theta2,
                        bias_lambda, bias_theta)
    res = run_bass_kernel_spmd(nc, maps, list(range(NCORES)))
    out = np.empty((B, N, E), np.float32)
    for c in range(NCORES):
        b, q = divmod(c, 4)
        out[b, q * NSLICE:(q + 1) * NSLICE] = res.results[c]["outb"]
    return out
